# revision 13
# baseline (speedup 1.0000x reference)
"""Trainium2 Bass kernel for nn_Poolinglabel_91104846282958.

The reference one-hots a [B,512,512] label map (19 classes) and runs seven
3x3 maxpools (strides 2,1,1,2,1,1,1).  Max of a one-hot mask over a window
chain is "class present in the effective receptive field": the cascade
composes to a single 39x39, stride-4, pad-19 window.  We encode each pixel
as an int32 bitmask (1<<class), OR-pool it separably (horizontal tree,
transpose, vertical tree), and decode 19 fp16 presence planes.

Pure data parallel: batch b -> NeuronCore b (B=8, 8 cores), no collectives.
"""
import sys

if "/opt/trn_rl_repo" not in sys.path:
    sys.path.insert(0, "/opt/trn_rl_repo")

import numpy as np

B = 8
R = 512          # rows
C = 512          # cols
S = 4            # row segments of 128
P = 128          # partitions
PADL = 19
W = 552          # PADL + 512 + 21 right pad
OC = 128         # output cols
ORR = 128        # output rows
NCLS = 19

LN2 = 0.6931471805599453
EXP_BIAS = 0.4 / (1 << 18)   # keeps exp result in [2^c+~0.1, 2^c+0.45]

_PROGRAM = None


def _or_tree(nc, eng, pool, src, seg0, nseg, dst, tag):
    """8-op OR-tree over nseg W-wide padded int32 segments of src, starting
    at segment seg0 (segments at stride W in the free dim):
    dst[:, s, o] = OR of src[:, (seg0+s)*W + 4o .. +4o+38], o in [0,128).
    dst: [P, nseg, 128] AP (any dtype). Bitwise ops are DVE-only."""
    import concourse.mybir as mybir

    I32 = mybir.dt.int32
    OR_ = mybir.AluOpType.bitwise_or

    uv = pool.tile([P, nseg * 2 * 137], I32, tag=f"uv{tag}")
    g = pool.tile([P, nseg * 136], I32, tag=f"g{tag}")
    d1 = pool.tile([P, nseg * 135], I32, tag=f"d1{tag}")
    s4 = pool.tile([P, nseg * 133], I32, tag=f"s4{tag}")
    z1 = pool.tile([P, nseg * 128], I32, tag=f"z1{tag}")
    z2 = pool.tile([P, nseg * 128], I32, tag=f"z2{tag}")
    z3 = pool.tile([P, nseg * 128], I32, tag=f"z3{tag}")

    base = src[:, seg0 * W : (seg0 + nseg) * W].rearrange("p (s w) -> p s w", w=W)

    def m4(elem_off, num):
        # [P, nseg, 2, num]: k in {0,1} selects src[.., 4j + elem_off + 2k]
        s0 = base[:, :, elem_off : elem_off + 4 * num]
        return (s0.rearrange("p s (a b) -> p s a b", b=4)[:, :, :, 0:3:2]
                .transpose([0, 1, 3, 2]))

    def seg(t, width, lo=0, hi=None):
        hi = width if hi is None else hi
        return t[:].rearrange("p (s w) -> p s w", w=width)[:, :, lo:hi]

    uv4 = uv[:].rearrange("p (s a b) -> p s a b", a=2, b=137)
    eng.tensor_tensor(uv4, m4(0, 137), m4(1, 137), OR_)   # u|v pairs
    uvv = uv[:].rearrange("p (s w) -> p s w", w=2 * 137)
    u = uvv[:, :, 0:137]
    v = uvv[:, :, 137 : 2 * 137]
    eng.tensor_tensor(seg(g, 136), u[:, :, 0:136], v[:, :, 0:136], OR_)      # 4j..4j+3
    eng.tensor_tensor(seg(d1, 135), seg(g, 136, 0, 135), seg(g, 136, 1, 136), OR_)
    eng.tensor_tensor(seg(s4, 133), seg(d1, 135, 0, 133), seg(d1, 135, 2, 135), OR_)
    eng.tensor_tensor(seg(z1, 128), seg(s4, 133, 0, 128), seg(s4, 133, 4, 132), OR_)
    eng.tensor_tensor(seg(z2, 128), seg(z1, 128), seg(g, 136, 8, 136), OR_)
    eng.tensor_tensor(seg(z3, 128), seg(z2, 128), u[:, :, 9:137], OR_)
    eng.tensor_tensor(dst, seg(z3, 128),
                      base[:, :, 38 : 38 + 4 * 127 + 1 : 4], OR_)


def _build_body(tc, y_d, x_d, dve_segs=(0, 1), dve_decode=12, use_exp_encode=False):
    import concourse.mybir as mybir

    nc = tc.nc
    F32 = mybir.dt.float32
    I32 = mybir.dt.int32
    F16 = mybir.dt.float16

    with tc.tile_pool(name="main", bufs=1) as pool, \
         tc.tile_pool(name="psum", bufs=2, space="PSUM") as psum:
        rid = pool.tile([P, P], F32)
        cid = pool.tile([P, P], F32)
        ident = pool.tile([P, P], F32)
        nc.gpsimd.iota(rid[:], [[0, P]], channel_multiplier=1,
                       allow_small_or_imprecise_dtypes=True)
        nc.gpsimd.iota(cid[:], [[1, P]], channel_multiplier=0,
                       allow_small_or_imprecise_dtypes=True)
        nc.vector.tensor_tensor(ident[:], rid[:], cid[:], mybir.AluOpType.is_equal)

        xin = pool.tile([P, S * C], F32)
        mbuf = pool.tile([P, S * W], I32)
        hbuf_i = pool.tile([P, S * OC], I32)
        hbuf = pool.tile([P, S * OC], F32)

        ebias = pool.tile([P, 1], F32)
        if use_exp_encode:
            nc.vector.memset(ebias[:], EXP_BIAS)

        mb3 = mbuf[:].rearrange("p (s w) -> p s w", w=W)
        nc.gpsimd.memset(mb3[:, :, 0:PADL], 0)
        nc.gpsimd.memset(mb3[:, :, PADL + C : W], 0)

        for s in range(S):
            nc.sync.dma_start(out=xin[:, s * C : (s + 1) * C],
                              in_=x_d[s * P : (s + 1) * P, :])

        for s in range(S):
            seg_in = xin[:, s * C : (s + 1) * C]
            seg_m = mbuf[:, s * W + PADL : s * W + PADL + C]
            if use_exp_encode:
                # int(exp(ln2*c + eps)) == 1<<c for c in [0,19)
                nc.scalar.activation(seg_m, seg_in, mybir.ActivationFunctionType.Exp,
                                     bias=ebias[:], scale=LN2)
            else:
                # K = 0x3F800000 + (c<<23) in f32 value space (exact);
                # bitcast K -> f32 has value 2^c; value-convert -> int32 = 1<<c
                kbuf = pool.tile([P, C], I32, tag=f"kb{s}")
                nc.scalar.activation(kbuf[:], seg_in, mybir.ActivationFunctionType.Copy,
                                     bias=1065353216.0, scale=8388608.0)
                nc.vector.tensor_copy(seg_m, kbuf[:].bitcast(F32))

        # horizontal trees: DVE-only (bitwise int32); 2 segs per op batch
        for s0 in range(0, S, 2):
            dst = (hbuf_i[:, s0 * OC : (s0 + 2) * OC]
                   .rearrange("p (s w) -> p s w", w=OC))
            _or_tree(nc, nc.vector, pool, mbuf, s0, 2, dst, tag=f"h{s0}")
            # value-convert int32 -> f32 for the PE transpose (ACT)
            nc.scalar.copy(hbuf[:, s0 * OC : (s0 + 2) * OC],
                           hbuf_i[:, s0 * OC : (s0 + 2) * OC])

        vbuf2 = pool.tile([P, W], I32)
        nc.gpsimd.memset(vbuf2[:, 0:PADL], 0)
        nc.gpsimd.memset(vbuf2[:, PADL + R : W], 0)
        for s in range(S):
            pt = psum.tile([P, P], F32, tag="pt")
            nc.tensor.transpose(pt[:], hbuf[:, s * OC : (s + 1) * OC], ident[:])
            nc.scalar.copy(vbuf2[:, PADL + s * P : PADL + (s + 1) * P], pt[:])

        obuf_i = pool.tile([P, ORR], I32)
        obuf = pool.tile([P, ORR], F32)
        _or_tree(nc, nc.vector, pool, vbuf2, 0, 1,
                 obuf_i[:].rearrange("p (s w) -> p s w", w=ORR), tag="v")
        nc.scalar.copy(obuf[:], obuf_i[:])

        ptf = psum.tile([P, P], F32, tag="pt")
        nc.tensor.transpose(ptf[:], obuf[:], ident[:])
        fbuf = pool.tile([P, P], I32)
        nc.scalar.copy(fbuf[:], ptf[:])

        # decode: (m >> c) & 1 as int32 (bitwise ops cannot cast), then one
        # ACT convert pass to fp16
        dec_i = pool.tile([P, NCLS * OC], I32)
        dec = pool.tile([P, NCLS * OC], F16)
        for c in range(NCLS):
            nc.vector.tensor_scalar(dec_i[:, c * OC : (c + 1) * OC], fbuf[:], c, 1,
                                    mybir.AluOpType.logical_shift_right,
                                    mybir.AluOpType.bitwise_and)
        nc.scalar.copy(dec[:], dec_i[:])

        for c in range(NCLS):
            nc.sync.dma_start(out=y_d[c], in_=dec[:, c * OC : (c + 1) * OC])


def _split_waits(nc, maxw=1):
    """The axon/walrus codegen path encodes at most one sync-wait per
    instruction; hoist excess waits onto preceding same-engine NoOps."""
    import concourse.mybir as mybir

    cnt = 0
    for fn in nc.m.functions:
        for blk in fn.blocks:
            newlist = []
            for inst in blk.instructions:
                si = inst.sync_info
                if si and si.on_wait and len(si.on_wait) > maxw:
                    waits = list(si.on_wait)
                    head, tail = waits[:-maxw], waits[-maxw:]
                    k = 0
                    while head:
                        chunk, head = head[:maxw], head[maxw:]
                        n = mybir.InstNoOp(name=f"{inst.name}-w{k}", ins=[], outs=[])
                        n.engine = inst.engine
                        n.sync_info = mybir.SyncInfo(on_wait=chunk, on_update=[])
                        newlist.append(n)
                        cnt += 1
                        k += 1
                    inst.sync_info = mybir.SyncInfo(on_wait=tail,
                                                    on_update=list(si.on_update or []))
                newlist.append(inst)
            blk.instructions[:] = newlist
    return cnt


def _build_program():
    global _PROGRAM
    if _PROGRAM is None:
        import concourse.bass as bass
        import concourse.mybir as mybir
        from concourse.tile import TileContext

        nc = bass.Bass("TRN2", debug=False)
        x_h = nc.declare_dram_parameter("x", [R, C], mybir.dt.float32, isOutput=False)
        y_h = nc.declare_dram_parameter("y", [NCLS, ORR, OC], mybir.dt.float16,
                                        isOutput=True)
        with TileContext(nc) as tc:
            _build_body(tc, y_h.ap(), x_h.ap())
        _split_waits(nc)
        _PROGRAM = nc
    return _PROGRAM


def kernel(x: np.ndarray) -> np.ndarray:
    """x: [8,512,512] float32 class ids -> [8,19,128,128] float16."""
    from concourse.bass_utils import run_bass_kernel_spmd

    nc = _build_program()
    x = np.ascontiguousarray(np.asarray(x, dtype=np.float32))
    assert x.shape == (B, R, C), x.shape
    in_maps = [{"x": x[i]} for i in range(B)]
    res = run_bass_kernel_spmd(nc, in_maps, list(range(B)))
    return np.stack([np.asarray(res.results[i]["y"], dtype=np.float16)
                     for i in range(B)])


# revision 15
# speedup vs baseline: 1.2668x; 1.2668x over previous
"""Trainium2 Bass kernel for nn_Poolinglabel_91104846282958.

The reference one-hots a [B,512,512] label map (19 classes) and runs seven
3x3 maxpools (strides 2,1,1,2,1,1,1).  Max of a one-hot mask over a window
chain is "class present in the effective receptive field": the cascade
composes to a single 39x39, stride-4, pad-19 window.  We encode each pixel
as an int32 bitmask (1<<class), OR-pool it separably (horizontal tree,
transpose, vertical tree), and decode 19 fp16 presence planes.

Pure data parallel: batch b -> NeuronCore b (B=8, 8 cores), no collectives.
"""
import sys

if "/opt/trn_rl_repo" not in sys.path:
    sys.path.insert(0, "/opt/trn_rl_repo")

import numpy as np

B = 8
R = 512          # rows
C = 512          # cols
S = 4            # row segments of 128
P = 128          # partitions
PADL = 19
W = 552          # PADL + 512 + 21 right pad
OC = 128         # output cols
ORR = 128        # output rows
NCLS = 19

LN2 = 0.6931471805599453
EXP_BIAS = 0.4 / (1 << 18)   # keeps exp result in [2^c+~0.1, 2^c+0.45]

_PROGRAM = None


def _or_tree(nc, eng, pool, src, seg0, nseg, dst, tag):
    """8-op OR-tree over nseg W-wide padded int32 segments of src, starting
    at segment seg0 (segments at stride W in the free dim):
    dst[:, s, o] = OR of src[:, (seg0+s)*W + 4o .. +4o+38], o in [0,128).
    dst: [P, nseg, 128] AP (any dtype). Bitwise ops are DVE-only."""
    import concourse.mybir as mybir

    I32 = mybir.dt.int32
    OR_ = mybir.AluOpType.bitwise_or

    uv = pool.tile([P, nseg * 2 * 137], I32, tag=f"uv{tag}")
    g = pool.tile([P, nseg * 136], I32, tag=f"g{tag}")
    d1 = pool.tile([P, nseg * 135], I32, tag=f"d1{tag}")
    s4 = pool.tile([P, nseg * 133], I32, tag=f"s4{tag}")
    z1 = pool.tile([P, nseg * 128], I32, tag=f"z1{tag}")
    z2 = pool.tile([P, nseg * 128], I32, tag=f"z2{tag}")
    z3 = pool.tile([P, nseg * 128], I32, tag=f"z3{tag}")

    base = src[:, seg0 * W : (seg0 + nseg) * W].rearrange("p (s w) -> p s w", w=W)

    def m4(elem_off, num):
        # [P, nseg, 2, num]: k in {0,1} selects src[.., 4j + elem_off + 2k]
        s0 = base[:, :, elem_off : elem_off + 4 * num]
        return (s0.rearrange("p s (a b) -> p s a b", b=4)[:, :, :, 0:3:2]
                .transpose([0, 1, 3, 2]))

    def seg(t, width, lo=0, hi=None):
        hi = width if hi is None else hi
        return t[:].rearrange("p (s w) -> p s w", w=width)[:, :, lo:hi]

    uv4 = uv[:].rearrange("p (s a b) -> p s a b", a=2, b=137)
    eng.tensor_tensor(uv4, m4(0, 137), m4(1, 137), OR_)   # u|v pairs
    uvv = uv[:].rearrange("p (s w) -> p s w", w=2 * 137)
    u = uvv[:, :, 0:137]
    v = uvv[:, :, 137 : 2 * 137]
    eng.tensor_tensor(seg(g, 136), u[:, :, 0:136], v[:, :, 0:136], OR_)      # 4j..4j+3
    eng.tensor_tensor(seg(d1, 135), seg(g, 136, 0, 135), seg(g, 136, 1, 136), OR_)
    eng.tensor_tensor(seg(s4, 133), seg(d1, 135, 0, 133), seg(d1, 135, 2, 135), OR_)
    eng.tensor_tensor(seg(z1, 128), seg(s4, 133, 0, 128), seg(s4, 133, 4, 132), OR_)
    eng.tensor_tensor(seg(z2, 128), seg(z1, 128), seg(g, 136, 8, 136), OR_)
    eng.tensor_tensor(seg(z3, 128), seg(z2, 128), u[:, :, 9:137], OR_)
    eng.tensor_tensor(dst, seg(z3, 128),
                      base[:, :, 38 : 38 + 4 * 127 + 1 : 4], OR_)


def _build_body(tc, y_d, x_d, dve_segs=(0, 1), dve_decode=12, use_exp_encode=False):
    import concourse.mybir as mybir

    nc = tc.nc
    F32 = mybir.dt.float32
    I32 = mybir.dt.int32
    F16 = mybir.dt.float16

    with tc.tile_pool(name="main", bufs=1) as pool, \
         tc.tile_pool(name="psum", bufs=2, space="PSUM") as psum:
        rid = pool.tile([P, P], F32)
        cid = pool.tile([P, P], F32)
        ident = pool.tile([P, P], F32)
        nc.gpsimd.iota(rid[:], [[0, P]], channel_multiplier=1,
                       allow_small_or_imprecise_dtypes=True)
        nc.gpsimd.iota(cid[:], [[1, P]], channel_multiplier=0,
                       allow_small_or_imprecise_dtypes=True)
        nc.vector.tensor_tensor(ident[:], rid[:], cid[:], mybir.AluOpType.is_equal)

        xin = pool.tile([P, S * C], F32)
        mbuf = pool.tile([P, S * W], I32)
        hbuf_i = pool.tile([P, S * OC], I32)
        hbuf = pool.tile([P, S * OC], F32)

        ebias = pool.tile([P, 1], F32)
        if use_exp_encode:
            nc.vector.memset(ebias[:], EXP_BIAS)

        mb3 = mbuf[:].rearrange("p (s w) -> p s w", w=W)
        nc.gpsimd.memset(mb3[:, :, 0:PADL], 0)
        nc.gpsimd.memset(mb3[:, :, PADL + C : W], 0)

        # two bulk input DMAs (each split across the 16 SDMA engines)
        for h in range(2):
            nc.sync.dma_start(
                out=xin[:, h * 2 * C : (h + 1) * 2 * C]
                    .rearrange("p (s c) -> p s c", c=C),
                in_=x_d[h * 2 * P : (h + 1) * 2 * P, :]
                    .rearrange("(s p) c -> p s c", p=P))

        for s in range(S):
            seg_in = xin[:, s * C : (s + 1) * C]
            seg_m = mbuf[:, s * W + PADL : s * W + PADL + C]
            # K = 0x3F800000 + (c<<23) in f32 value space (exact), out int32;
            # bitcast K -> f32 has value 2^c; value-convert -> int32 = 1<<c
            kbuf = pool.tile([P, C], I32, tag=f"kb{s}")
            nc.gpsimd.tensor_scalar(kbuf[:], seg_in, 8388608.0, 1065353216.0,
                                    mybir.AluOpType.mult, mybir.AluOpType.add)
            nc.gpsimd.tensor_copy(seg_m, kbuf[:].bitcast(F32))

        # horizontal trees: DVE-only (bitwise int32); 2 segs per op batch
        for s0 in range(0, S, 2):
            dst = (hbuf_i[:, s0 * OC : (s0 + 2) * OC]
                   .rearrange("p (s w) -> p s w", w=OC))
            _or_tree(nc, nc.vector, pool, mbuf, s0, 2, dst, tag=f"h{s0}")
            # value-convert int32 -> f32 for the PE transpose (ACT)
            nc.scalar.copy(hbuf[:, s0 * OC : (s0 + 2) * OC],
                           hbuf_i[:, s0 * OC : (s0 + 2) * OC])

        vbuf2 = pool.tile([P, W], I32)
        nc.gpsimd.memset(vbuf2[:, 0:PADL], 0)
        nc.gpsimd.memset(vbuf2[:, PADL + R : W], 0)
        for s in range(S):
            pt = psum.tile([P, P], F32, tag="pt")
            nc.tensor.transpose(pt[:], hbuf[:, s * OC : (s + 1) * OC], ident[:])
            nc.scalar.copy(vbuf2[:, PADL + s * P : PADL + (s + 1) * P], pt[:])

        obuf_i = pool.tile([P, ORR], I32)
        obuf = pool.tile([P, ORR], F32)
        _or_tree(nc, nc.vector, pool, vbuf2, 0, 1,
                 obuf_i[:].rearrange("p (s w) -> p s w", w=ORR), tag="v")
        nc.scalar.copy(obuf[:], obuf_i[:])

        ptf = psum.tile([P, P], F32, tag="pt")
        nc.tensor.transpose(ptf[:], obuf[:], ident[:])
        fbuf = pool.tile([P, P], I32)
        nc.scalar.copy(fbuf[:], ptf[:])

        # decode: (m >> c) & 1 as int32 (bitwise ops cannot cast), then
        # convert to fp16 in two halves, each followed by its bulk DMA out
        dec_i = pool.tile([P, NCLS * OC], I32)
        dec = pool.tile([P, NCLS * OC], F16)
        for c in range(NCLS):
            nc.vector.tensor_scalar(dec_i[:, c * OC : (c + 1) * OC], fbuf[:], c, 1,
                                    mybir.AluOpType.logical_shift_right,
                                    mybir.AluOpType.bitwise_and)
        for (c0, c1) in ((0, 10), (10, NCLS)):
            nc.scalar.copy(dec[:, c0 * OC : c1 * OC], dec_i[:, c0 * OC : c1 * OC])
            nc.sync.dma_start(
                out=y_d[c0:c1].rearrange("c p w -> p c w"),
                in_=dec[:, c0 * OC : c1 * OC].rearrange("p (c w) -> p c w", w=OC))


def _split_waits(nc, maxw=1):
    """The axon/walrus codegen path encodes at most one sync-wait per
    instruction; hoist excess waits onto preceding same-engine NoOps."""
    import concourse.mybir as mybir

    cnt = 0
    for fn in nc.m.functions:
        for blk in fn.blocks:
            newlist = []
            for inst in blk.instructions:
                si = inst.sync_info
                if si and si.on_wait and len(si.on_wait) > maxw:
                    waits = list(si.on_wait)
                    head, tail = waits[:-maxw], waits[-maxw:]
                    k = 0
                    while head:
                        chunk, head = head[:maxw], head[maxw:]
                        n = mybir.InstNoOp(name=f"{inst.name}-w{k}", ins=[], outs=[])
                        n.engine = inst.engine
                        n.sync_info = mybir.SyncInfo(on_wait=chunk, on_update=[])
                        newlist.append(n)
                        cnt += 1
                        k += 1
                    inst.sync_info = mybir.SyncInfo(on_wait=tail,
                                                    on_update=list(si.on_update or []))
                newlist.append(inst)
            blk.instructions[:] = newlist
    return cnt


def _build_program():
    global _PROGRAM
    if _PROGRAM is None:
        import concourse.bass as bass
        import concourse.mybir as mybir
        from concourse.tile import TileContext

        nc = bass.Bass("TRN2", debug=False)
        x_h = nc.declare_dram_parameter("x", [R, C], mybir.dt.float32, isOutput=False)
        y_h = nc.declare_dram_parameter("y", [NCLS, ORR, OC], mybir.dt.float16,
                                        isOutput=True)
        with TileContext(nc) as tc:
            _build_body(tc, y_h.ap(), x_h.ap())
        _split_waits(nc)
        _PROGRAM = nc
    return _PROGRAM


def kernel(x: np.ndarray) -> np.ndarray:
    """x: [8,512,512] float32 class ids -> [8,19,128,128] float16."""
    from concourse.bass_utils import run_bass_kernel_spmd

    nc = _build_program()
    x = np.ascontiguousarray(np.asarray(x, dtype=np.float32))
    assert x.shape == (B, R, C), x.shape
    in_maps = [{"x": x[i]} for i in range(B)]
    res = run_bass_kernel_spmd(nc, in_maps, list(range(B)))
    return np.stack([np.asarray(res.results[i]["y"], dtype=np.float16)
                     for i in range(B)])


# revision 17
# speedup vs baseline: 1.4857x; 1.1728x over previous
"""Trainium2 Bass kernel for nn_Poolinglabel_91104846282958.

The reference one-hots a [B,512,512] label map (19 classes) and runs seven
3x3 maxpools (strides 2,1,1,2,1,1,1).  Max of a one-hot mask over a window
chain is "class present in the effective receptive field": the cascade
composes to a single 39x39, stride-4, pad-19 window.  We encode each pixel
as an int32 bitmask (1<<class), OR-pool it separably (horizontal tree,
transpose, vertical tree), and decode 19 fp16 presence planes.

Pure data parallel: batch b -> NeuronCore b (B=8, 8 cores), no collectives.
"""
import sys

if "/opt/trn_rl_repo" not in sys.path:
    sys.path.insert(0, "/opt/trn_rl_repo")

import numpy as np

B = 8
R = 512          # rows
C = 512          # cols
S = 4            # row segments of 128
P = 128          # partitions
PADL = 19
W = 552          # PADL + 512 + 21 right pad
OC = 128         # output cols
ORR = 128        # output rows
NCLS = 19

LN2 = 0.6931471805599453
EXP_BIAS = 0.4 / (1 << 18)   # keeps exp result in [2^c+~0.1, 2^c+0.45]

_PROGRAM = None


def _or_tree(nc, eng, pool, src, seg0, nseg, dst, tag):
    """8-op OR-tree over nseg W-wide padded int32 segments of src, starting
    at segment seg0 (segments at stride W in the free dim):
    dst[:, s, o] = OR of src[:, (seg0+s)*W + 4o .. +4o+38], o in [0,128).
    dst: [P, nseg, 128] AP (any dtype). Bitwise ops are DVE-only."""
    import concourse.mybir as mybir

    I32 = mybir.dt.int32
    OR_ = mybir.AluOpType.bitwise_or

    uv = pool.tile([P, nseg * 2 * 137], I32, tag=f"uv{tag}")
    g = pool.tile([P, nseg * 136], I32, tag=f"g{tag}")
    d1 = pool.tile([P, nseg * 135], I32, tag=f"d1{tag}")
    s4 = pool.tile([P, nseg * 133], I32, tag=f"s4{tag}")
    z1 = pool.tile([P, nseg * 128], I32, tag=f"z1{tag}")
    z2 = pool.tile([P, nseg * 128], I32, tag=f"z2{tag}")
    z3 = pool.tile([P, nseg * 128], I32, tag=f"z3{tag}")

    base = src[:, seg0 * W : (seg0 + nseg) * W].rearrange("p (s w) -> p s w", w=W)

    def m4(elem_off, num):
        # [P, nseg, 2, num]: k in {0,1} selects src[.., 4j + elem_off + 2k]
        s0 = base[:, :, elem_off : elem_off + 4 * num]
        return (s0.rearrange("p s (a b) -> p s a b", b=4)[:, :, :, 0:3:2]
                .transpose([0, 1, 3, 2]))

    def seg(t, width, lo=0, hi=None):
        hi = width if hi is None else hi
        return t[:].rearrange("p (s w) -> p s w", w=width)[:, :, lo:hi]

    uv4 = uv[:].rearrange("p (s a b) -> p s a b", a=2, b=137)
    eng.tensor_tensor(uv4, m4(0, 137), m4(1, 137), OR_)   # u|v pairs
    uvv = uv[:].rearrange("p (s w) -> p s w", w=2 * 137)
    u = uvv[:, :, 0:137]
    v = uvv[:, :, 137 : 2 * 137]
    eng.tensor_tensor(seg(g, 136), u[:, :, 0:136], v[:, :, 0:136], OR_)      # 4j..4j+3
    eng.tensor_tensor(seg(d1, 135), seg(g, 136, 0, 135), seg(g, 136, 1, 136), OR_)
    eng.tensor_tensor(seg(s4, 133), seg(d1, 135, 0, 133), seg(d1, 135, 2, 135), OR_)
    eng.tensor_tensor(seg(z1, 128), seg(s4, 133, 0, 128), seg(s4, 133, 4, 132), OR_)
    eng.tensor_tensor(seg(z2, 128), seg(z1, 128), seg(g, 136, 8, 136), OR_)
    eng.tensor_tensor(seg(z3, 128), seg(z2, 128), u[:, :, 9:137], OR_)
    eng.tensor_tensor(dst, seg(z3, 128),
                      base[:, :, 38 : 38 + 4 * 127 + 1 : 4], OR_)


def _build_body(tc, y_d, x_d, dve_segs=(0, 1), dve_decode=12, use_exp_encode=False):
    import concourse.mybir as mybir

    nc = tc.nc
    F32 = mybir.dt.float32
    I32 = mybir.dt.int32
    F16 = mybir.dt.float16

    with tc.tile_pool(name="main", bufs=1) as pool, \
         tc.tile_pool(name="psum", bufs=2, space="PSUM") as psum:
        rid = pool.tile([P, P], F32)
        cid = pool.tile([P, P], F32)
        ident = pool.tile([P, P], F32)
        nc.gpsimd.iota(rid[:], [[0, P]], channel_multiplier=1,
                       allow_small_or_imprecise_dtypes=True)
        nc.gpsimd.iota(cid[:], [[1, P]], channel_multiplier=0,
                       allow_small_or_imprecise_dtypes=True)
        nc.vector.tensor_tensor(ident[:], rid[:], cid[:], mybir.AluOpType.is_equal)

        xin = pool.tile([P, S * C], F32)
        mbuf = pool.tile([P, S * W], I32)
        hbuf_i = pool.tile([P, S * OC], I32)
        hbuf = pool.tile([P, S * OC], F32)

        ebias = pool.tile([P, 1], F32)
        if use_exp_encode:
            nc.vector.memset(ebias[:], EXP_BIAS)

        mb3 = mbuf[:].rearrange("p (s w) -> p s w", w=W)
        nc.gpsimd.memset(mb3[:, :, 0:PADL], 0)
        nc.gpsimd.memset(mb3[:, :, PADL + C : W], 0)

        # per-segment input DMAs alternating between the SP and ACT HWDGE rings
        for s in range(S):
            eng = nc.sync if s % 2 == 0 else nc.scalar
            eng.dma_start(out=xin[:, s * C : (s + 1) * C],
                          in_=x_d[s * P : (s + 1) * P, :])

        for s in range(S):
            seg_in = xin[:, s * C : (s + 1) * C]
            seg_m = mbuf[:, s * W + PADL : s * W + PADL + C]
            # K = 0x3F800000 + (c<<23) in f32 value space (exact), out int32;
            # bitcast K -> f32 has value 2^c; value-convert -> int32 = 1<<c
            kbuf = pool.tile([P, C], I32, tag=f"kb{s}")
            nc.gpsimd.tensor_scalar(kbuf[:], seg_in, 8388608.0, 1065353216.0,
                                    mybir.AluOpType.mult, mybir.AluOpType.add)
            nc.scalar.copy(seg_m, kbuf[:].bitcast(F32))

        # horizontal trees: DVE-only (bitwise int32); 2 segs per op batch
        for s0 in range(0, S, 2):
            dst = (hbuf_i[:, s0 * OC : (s0 + 2) * OC]
                   .rearrange("p (s w) -> p s w", w=OC))
            _or_tree(nc, nc.vector, pool, mbuf, s0, 2, dst, tag=f"h{s0}")
            # value-convert int32 -> f32 for the PE transpose (ACT)
            nc.scalar.copy(hbuf[:, s0 * OC : (s0 + 2) * OC],
                           hbuf_i[:, s0 * OC : (s0 + 2) * OC])

        vbuf2 = pool.tile([P, W], I32)
        nc.gpsimd.memset(vbuf2[:, 0:PADL], 0)
        nc.gpsimd.memset(vbuf2[:, PADL + R : W], 0)
        for s in range(S):
            pt = psum.tile([P, P], F32, tag="pt")
            nc.tensor.transpose(pt[:], hbuf[:, s * OC : (s + 1) * OC], ident[:])
            nc.scalar.copy(vbuf2[:, PADL + s * P : PADL + (s + 1) * P], pt[:])

        obuf_i = pool.tile([P, ORR], I32)
        obuf = pool.tile([P, ORR], F32)
        _or_tree(nc, nc.vector, pool, vbuf2, 0, 1,
                 obuf_i[:].rearrange("p (s w) -> p s w", w=ORR), tag="v")
        nc.scalar.copy(obuf[:], obuf_i[:])

        ptf = psum.tile([P, P], F32, tag="pt")
        nc.tensor.transpose(ptf[:], obuf[:], ident[:])
        fbuf = pool.tile([P, P], I32)
        nc.scalar.copy(fbuf[:], ptf[:])

        # decode: (m >> c) & 1 as int32 (bitwise ops cannot cast), then
        # convert to fp16 in two halves, each followed by its bulk DMA out
        dec_i = pool.tile([P, NCLS * OC], I32)
        dec = pool.tile([P, NCLS * OC], F16)
        for c in range(NCLS):
            nc.vector.tensor_scalar(dec_i[:, c * OC : (c + 1) * OC], fbuf[:], c, 1,
                                    mybir.AluOpType.logical_shift_right,
                                    mybir.AluOpType.bitwise_and)
        for k, (c0, c1) in enumerate(((0, 10), (10, NCLS))):
            nc.scalar.copy(dec[:, c0 * OC : c1 * OC], dec_i[:, c0 * OC : c1 * OC])
            deng = nc.sync if k == 0 else nc.scalar
            deng.dma_start(
                out=y_d[c0:c1].rearrange("c p w -> p c w"),
                in_=dec[:, c0 * OC : c1 * OC].rearrange("p (c w) -> p c w", w=OC))


def _split_waits(nc, maxw=1):
    """The axon/walrus codegen path encodes at most one sync-wait per
    instruction; hoist excess waits onto preceding same-engine NoOps."""
    import concourse.mybir as mybir

    cnt = 0
    for fn in nc.m.functions:
        for blk in fn.blocks:
            newlist = []
            for inst in blk.instructions:
                si = inst.sync_info
                if si and si.on_wait and len(si.on_wait) > maxw:
                    waits = list(si.on_wait)
                    head, tail = waits[:-maxw], waits[-maxw:]
                    k = 0
                    while head:
                        chunk, head = head[:maxw], head[maxw:]
                        n = mybir.InstNoOp(name=f"{inst.name}-w{k}", ins=[], outs=[])
                        n.engine = inst.engine
                        n.sync_info = mybir.SyncInfo(on_wait=chunk, on_update=[])
                        newlist.append(n)
                        cnt += 1
                        k += 1
                    inst.sync_info = mybir.SyncInfo(on_wait=tail,
                                                    on_update=list(si.on_update or []))
                newlist.append(inst)
            blk.instructions[:] = newlist
    return cnt


def _build_program():
    global _PROGRAM
    if _PROGRAM is None:
        import concourse.bass as bass
        import concourse.mybir as mybir
        from concourse.tile import TileContext

        nc = bass.Bass("TRN2", debug=False)
        x_h = nc.declare_dram_parameter("x", [R, C], mybir.dt.float32, isOutput=False)
        y_h = nc.declare_dram_parameter("y", [NCLS, ORR, OC], mybir.dt.float16,
                                        isOutput=True)
        with TileContext(nc) as tc:
            _build_body(tc, y_h.ap(), x_h.ap())
        _split_waits(nc)
        _PROGRAM = nc
    return _PROGRAM


def kernel(x: np.ndarray) -> np.ndarray:
    """x: [8,512,512] float32 class ids -> [8,19,128,128] float16."""
    from concourse.bass_utils import run_bass_kernel_spmd

    nc = _build_program()
    x = np.ascontiguousarray(np.asarray(x, dtype=np.float32))
    assert x.shape == (B, R, C), x.shape
    in_maps = [{"x": x[i]} for i in range(B)]
    res = run_bass_kernel_spmd(nc, in_maps, list(range(B)))
    return np.stack([np.asarray(res.results[i]["y"], dtype=np.float16)
                     for i in range(B)])
